# revision 8
# baseline (speedup 1.0000x reference)
"""Trainium2 Bass kernel for nn_CompProbModel_42691974922925.

Reference semantics: for each batch frame, the model builds a completion-
probability field over F=6600 field cells x NT=40 pass durations x P=10
players, then gathers a single row ``out = ind_pass[b_idx, tof, :]`` where
``b_idx`` (ball target cell) and ``tof`` (time-of-flight index) are scalars
derived from the frame. Exact dead-code elimination: the gathered row only
depends on the 40 trajectory cells ``path[b_idx, tof, s]`` (s = traj step),
so the live computation is a [40 steps x 10 players] problem:

    p[s,p]    = sigmoid(c * (T[tt_idx[s]] - t_tot(cell_s, player_p))) * lam_z[tof,s]
    q[s]      = max(1, sum_p p[s,p]);  pn = p / q
    all_t[s]  = sum_p pn[s,p]
    rem       = cumprod_s(1 - all_t);  shift = roll(rem, 1), shift[0] = 1
    out[p]    = sum_{s<=tof} shift[s] * pn[s,p] * lam_all[p]

Host side (numpy, f32-exact vs the jax reference): index math (tof, b_idx,
trajectory cell indices via round-half-even), gathering FIELD_LOCS rows and
packing operand blocks. Device side (Bass/Tile, per core): all the real
arithmetic - kinematics distances, sqrt/sigmoid, normalization, the exact
cumprod survival scan, and the final contraction as a PE matvec.

Device-side structure (all engines see a [P=10 partitions, 40 free] layout):

- Both square roots and the sigmoid run off ONE activation-function table
  (natural_log_exp_and_others): sqrt(x) = exp(0.5*ln(x)), 1/sqrt(d2) =
  exp(-0.5*ln(d2)) (which also removes the reciprocal for s0), and
  sigmoid(x) = 1/(1+exp(-x)). The single table load overlaps the input DMA
  instead of stalling the activation engine mid-kernel.
- The time-to-target math is algebraically compressed: in the
  speed-limited branch  t_tot - reax = tlt + (dmag-dlt)/sm  collapses to
  dmag/sm + (sm-s0)^2/(2*am*sm), and the branch condition d_lt > d_mag is
  exactly w1 < sm^2 where w1 = s0^2 + 2*am*dmag is the operand of the
  second sqrt - eliminating the tlt/hb/dlt/ee intermediate tensors.
- The catchability window lam_z folds into the host-packed time row:
  masked lanes get tgr = -1e30, so exp overflows to inf and the sigmoid
  underflows to exactly 0 (matching lam_z * p == 0).
- The player sum uses one PE matmul against an all-ones [P,P] block, which
  leaves the row sum REPLICATED across all partitions - the normalization,
  survival cumprod scan and shifted-mask weighting then stay elementwise
  on DVE and no second matmul / broadcast trip through PSUM is needed.

Sharding across the 8 NeuronCores: the live problem after the trajectory
reduction is tiny and sequential (cumprod over s), so inputs are replicated
and every core computes the full result redundantly; core 0's output is
returned. (The [F,40,40,P] field sweep the sharding hint describes is dead
code for the final gather, so there is nothing left worth splitting.)
"""

import numpy as np

f32 = np.float32
NX, NY, NT, P = 120, 55, 40, 10
F = NX * NY
G = 10.72468

# T_GRID = jnp.linspace(0.1, 4.0, 40, dtype=float32) - exact bits as produced
# by jax (identical on the CPU and neuron backends; np.linspace differs by
# 1 ulp at 6 entries, so the bit pattern is pinned here).
_TGRID_BITS = [
    0x3DCCCCCD, 0x3E4CCCCD, 0x3E99999A, 0x3ECCCCCD, 0x3F000000, 0x3F19999A,
    0x3F333334, 0x3F4CCCCD, 0x3F666667, 0x3F800000, 0x3F8CCCCD, 0x3F99999A,
    0x3FA66667, 0x3FB33334, 0x3FC00000, 0x3FCCCCCD, 0x3FD9999A, 0x3FE66666,
    0x3FF33333, 0x40000000, 0x40066667, 0x400CCCCD, 0x40133334, 0x4019999A,
    0x40200000, 0x40266667, 0x402CCCCD, 0x40333334, 0x4039999A, 0x40400000,
    0x40466667, 0x404CCCCD, 0x40533333, 0x4059999A, 0x40600000, 0x40666666,
    0x406CCCCD, 0x40733333, 0x4079999A, 0x40800000,
]
T_GRID = np.array(_TGRID_BITS, dtype=np.uint32).view(f32)

_x = np.linspace(0.5, 119.5, 120)
_y = np.linspace(-0.5, 53.5, 55)
_y[0] = -0.2
_yy, _xx = np.meshgrid(_y, _x, indexing="ij")
FIELD_LOCS = np.stack([_xx, _yy], -1).reshape(-1, 2).astype(f32)  # [6600,2]

# tt_idx[s] = round(10*T[s]-1): traj-step -> p_int time index (== arange(40)
# for these bits, but computed generically).
TT_IDX = np.round(f32(10.0) * T_GRID - f32(1.0)).astype(np.int32)

N_CORES = 8

# single packed device input [P, XCOLS] (f32); column map:
_C_CXY = 0      # 0:80    field-cell coords along traj (cx | cy), bcast over p
_C_RVXY = 80    # 80:160  player reaction velocities (rvx*40 | rvy*40)
_C_RLXY = 160   # 160:240 player reaction locations (rlx*40 | rly*40)
_C_TG = 240     # 240:280 tgr[s] = T[tt_idx[s]] - reax, or -1e30 on lam_z==0
_C_MA = 280     # 280:320 shift-mask row: col0 = msk[0]*mlz[0] (shift[0]==1
                #         lane), cols 1.. = msk*mlz; becomes w2 in place
_C_ONES = 320   # 320:330 ones (the [P,P] block for the PE broadcast-sum)
_C_SC = 330     # 330:340 sm, negsm, inv_am, neg_inv_am, two_am, neg_inv_sm,
                #         sqrt_c1, smsq, neg_sigc, lam
XCOLS = 344

_CACHE = {}


def _compile_with_lnexp_table(nc):
    """Compile with the act-table pass steered to the one set that holds
    both ln and exp (natural_log_exp_and_others), so the whole kernel runs
    off a single table load instead of reloading between every ln/exp.
    The pass picks the first set containing each activation's function, so
    hide the functions of every other set (ids stay aligned with
    act_info.json; only the search is narrowed). Falls back to the stock
    tables if the arch's table list doesn't match this layout."""
    import concourse.bacc as bacc
    import concourse.mybir as mybir

    orig = bacc.get_activation_tables
    want = {mybir.ActivationFunctionType.Ln, mybir.ActivationFunctionType.Exp}

    def patched(arch):
        tabs = list(orig(arch).items())
        if len(tabs) > 6 and want <= tabs[6][1]:
            return {name: (funcs if i == 6 else set())
                    for i, (name, funcs) in enumerate(tabs)}
        return dict(tabs)

    bacc.get_activation_tables = patched
    try:
        nc.compile()
    finally:
        bacc.get_activation_tables = orig


def _build_program():
    """Build + compile the 8-core Bass program once per process."""
    import concourse.bacc as bacc
    import concourse.bass as bass
    import concourse.mybir as mybir
    import concourse.tile as tile

    dt = mybir.dt.float32
    op = mybir.AluOpType
    act = mybir.ActivationFunctionType

    nc = bacc.Bacc("TRN2", target_bir_lowering=False, debug=False,
                   num_devices=N_CORES)
    x_dram = nc.dram_tensor("xin", [P, XCOLS], dt, kind="ExternalInput")
    out_dram = nc.dram_tensor("res", [P, 1], dt, kind="ExternalOutput")

    with tile.TileContext(nc) as tc:
        with (
            tc.tile_pool(name="sb", bufs=1) as pool,
            tc.tile_pool(name="ps", bufs=1, space=bass.MemorySpace.PSUM) as psp,
        ):
            x = pool.tile([P, XCOLS], dt, name="x", tag="x")
            nc.sync.dma_start(x[:], x_dram.ap())

            cxy = x[:, _C_CXY:_C_CXY + 80]
            rvxy = x[:, _C_RVXY:_C_RVXY + 80]
            rlxy = x[:, _C_RLXY:_C_RLXY + 80]
            tgr = x[:, _C_TG:_C_TG + 40]
            w2x = x[:, _C_MA:_C_MA + 40]
            ones_pp = x[:, _C_ONES:_C_ONES + P]
            sco = _C_SC
            sm, negsm = x[:, sco:sco + 1], x[:, sco + 1:sco + 2]
            inv_am, neg_inv_am = x[:, sco + 2:sco + 3], x[:, sco + 3:sco + 4]
            two_am, neg_inv_sm = x[:, sco + 4:sco + 5], x[:, sco + 5:sco + 6]
            sqrt_c1, smsq = x[:, sco + 6:sco + 7], x[:, sco + 7:sco + 8]
            neg_sigc, lam = x[:, sco + 8:sco + 9], x[:, sco + 9:sco + 10]

            def wt(name, p=P, n=NT):
                return pool.tile([p, n], dt, name=name, tag=name)

            # kinematics: dxy = cells - rloc; d2 goes to the activation
            # engine ASAP (it gates the ln/exp sqrt chain); the velocity
            # dot product is emitted later so the scheduler keeps it out of
            # the d2 chain and it fills the activation-wait window.
            dxy = wt("dxy", n=80)
            nc.vector.tensor_tensor(dxy[:], cxy, rlxy, op.subtract)
            sq = wt("sq", n=80)
            nc.vector.tensor_tensor(sq[:], dxy[:], dxy[:], op.mult)
            d2 = wt("d2")
            nc.vector.tensor_tensor(d2[:], sq[:, 0:40], sq[:, 40:80], op.add)
            # invd = exp(-0.5*ln(d2)) = 1/dmag ; dmag = exp(+0.5*ln(d2)).
            l2, invd, dmag = wt("l2"), wt("invd"), wt("dmag")
            nc.scalar.activation(l2[:], d2[:], act.Ln)
            nc.scalar.activation(invd[:], l2[:], act.Exp, scale=-0.5)
            nc.scalar.activation(dmag[:], l2[:], act.Exp, scale=0.5)
            # velocity dot (off the critical chain, runs during the ACT ops)
            nm = wt("nm", n=80)
            nc.vector.tensor_tensor(nm[:], dxy[:], rvxy, op.mult)
            num = wt("num")
            nc.vector.tensor_tensor(num[:], nm[:, 0:40], nm[:, 40:80], op.add)

            s0 = wt("s0")
            nc.vector.tensor_tensor(s0[:], num[:], invd[:], op.mult)
            nc.vector.tensor_scalar(s0[:], s0[:], sm, negsm, op.min, op.max)
            # speed-limited branch: pm0 = tgr - dmag/sm - ((s0-sm)*sqrt_c1)^2
            # with sqrt_c1 = sqrt(1/(2*am*sm)); accel-limited branch:
            # pm_alt = tgr - (rt - s0)/am with rt = sqrt(s0^2 + 2*am*dmag).
            s0sq, w1 = wt("s0sq"), wt("w1")
            nc.vector.tensor_tensor(s0sq[:], s0[:], s0[:], op.mult)
            nc.vector.scalar_tensor_tensor(w1[:], dmag[:], two_am, s0sq[:],
                                           op.mult, op.add)
            lw, rt = wt("lw"), wt("rt")
            nc.scalar.activation(lw[:], w1[:], act.Ln)
            nc.scalar.activation(rt[:], lw[:], act.Exp, scale=0.5)
            # the speed-limited pm and the branch mask fill the ACT window
            z1, pm, q1 = wt("z1"), wt("pm"), wt("q1")
            nc.vector.tensor_scalar(z1[:], s0[:], sm, sqrt_c1,
                                    op.subtract, op.mult)
            nc.vector.tensor_tensor(z1[:], z1[:], z1[:], op.mult)
            nc.vector.scalar_tensor_tensor(q1[:], dmag[:], neg_inv_sm, tgr,
                                           op.mult, op.add)
            nc.vector.tensor_tensor(pm[:], q1[:], z1[:], op.subtract)
            gm = pool.tile([P, NT], mybir.dt.uint8, name="gm", tag="gm")
            nc.vector.tensor_scalar(gm[:], w1[:], smsq, None, op.is_lt)
            h = wt("h")
            nc.vector.scalar_tensor_tensor(h[:], s0[:], inv_am, tgr,
                                           op.mult, op.add)
            pma = wt("pma")
            nc.vector.scalar_tensor_tensor(pma[:], rt[:], neg_inv_am, h[:],
                                           op.mult, op.add)
            nc.vector.copy_predicated(pm[:], gm[:], pma[:])
            # sigmoid(sigc*pm) = 1/(1+exp(-sigc*pm)); masked lanes carry
            # tgr=-1e30 so pex overflows to inf and pp becomes exactly 0.
            pex, den, pp = wt("pex"), wt("den"), wt("pp")
            nc.scalar.activation(pex[:], pm[:], act.Exp, scale=neg_sigc)
            nc.vector.tensor_scalar(den[:], pex[:], 1.0, None, op.add)
            nc.vector.reciprocal(pp[:], den[:])

            # player sum, replicated to every partition in one matmul
            ps_sum = psp.tile([P, NT], dt, name="ps_sum", tag="ps_sum")
            nc.tensor.matmul(ps_sum[:], ones_pp, pp[:])
            # survival factor: (max(1,S)-S)/max(1,S) == relu(1-S) exactly
            # (S>1 gives 0 via 0*invq, S<=1 gives (1-S)*1), so it comes
            # straight off PSUM on the otherwise-idle activation engine
            # while DVE runs the q = max(1,S) reciprocal chain.
            v = wt("v")
            nc.scalar.activation(v[:], ps_sum[:], act.Relu,
                                 bias=1.0, scale=-1.0)
            qrow, invq = wt("qrow"), wt("invq")
            nc.vector.tensor_scalar(qrow[:], ps_sum[:], 1.0, None, op.max)
            nc.vector.reciprocal(invq[:], qrow[:])
            # rem = cumprod(v); the shifted tril*lam_z mask row (host-packed
            # with the shift[0]==1 lane already in col 0) turns into w2 in
            # place: w2[s] = rem[s-1]*msk[s]*mlz[s], w2[0] = msk[0]*mlz[0].
            rem = wt("rem")
            nc.vector.tensor_tensor_scan(rem[:], v[:], v[:], 1.0,
                                         op.mult, op.bypass)
            nc.vector.tensor_tensor(w2x[:, 1:NT], rem[:, 0:NT - 1],
                                    w2x[:, 1:NT], op.mult)
            # final contraction: res[p] = sum_s (pp*invq)[p,s]*lam[p]*w2[p,s]
            u, ind = wt("u"), wt("ind")
            nc.vector.tensor_tensor(u[:], pp[:], invq[:], op.mult)
            res = pool.tile([P, 1], dt, name="res", tag="res")
            nc.vector.scalar_tensor_tensor(ind[:], u[:], lam, w2x,
                                           op.mult, op.mult,
                                           accum_out=res[:])
            nc.sync.dma_start(out_dram.ap(), res[:])

    _compile_with_lnexp_table(nc)
    return nc


def _get_nc():
    if "nc" not in _CACHE:
        _CACHE["nc"] = _build_program()
    return _CACHE["nc"]


def _host_prep(frame, s_max, a_max, tti_sigma, tti_lambda_off, tti_lambda_def):
    """Index math + operand packing for one batch element (numpy, f32)."""
    fr = np.asarray(frame, dtype=f32)[0]          # [P,13]
    sm = f32(np.asarray(s_max).reshape(-1)[0])
    am = f32(np.asarray(a_max).reshape(-1)[0])
    sig = f32(np.asarray(tti_sigma).reshape(-1)[0])
    lo = f32(np.asarray(tti_lambda_off).reshape(-1)[0])
    ld = f32(np.asarray(tti_lambda_def).reshape(-1)[0])

    reax = f32(sm / am)
    v_x_r = fr[:, 5] * reax + fr[:, 3]
    v_y_r = fr[:, 6] * reax + fr[:, 4]
    x_r = fr[:, 1] + fr[:, 3] * reax + f32(0.5) * fr[:, 5] * f32(reax * reax)
    y_r = fr[:, 2] + fr[:, 4] * reax + f32(0.5) * fr[:, 6] * f32(reax * reax)
    teams = fr[:, 7]
    rlx = x_r.astype(np.int32).astype(f32)        # trunc-toward-zero like jax
    rly = y_r.astype(np.int32).astype(f32)

    # scalar gathers (match jax negative-index wrap + OOB clip semantics)
    tof = int(np.round(fr[0, 12])) - 1
    if tof < 0:
        tof += NT
    tof = min(max(tof, 0), NT - 1)
    b_idx = (int(fr[0, 11]) + 1) * NX + int(fr[0, 10])
    if b_idx < 0:
        b_idx += F
    b_idx = min(max(b_idx, 0), F - 1)

    # ball trajectory for the (b_idx, tof) row; round-half-even like jnp.round
    ball = fr[0, 8:10]
    vx = f32((FIELD_LOCS[b_idx, 0] - ball[0]) / T_GRID[tof])
    vy = f32((FIELD_LOCS[b_idx, 1] - ball[1]) / T_GRID[tof])
    traj_x = np.round(
        np.minimum(np.maximum(ball[0] + vx * T_GRID, f32(0)), f32(NX - 1))
    ).astype(np.int32)
    traj_y = np.round(
        np.minimum(np.maximum(ball[1] + vy * T_GRID, f32(0)), f32(NY - 1))
    ).astype(np.int32)
    path = traj_y * NX + traj_x                    # [40] in-range by clip
    cells = FIELD_LOCS[path]                       # [40,2]

    # catchability window lam_z[tof, s]
    vz0_t = f32(T_GRID[tof] * f32(G) / f32(2.0))
    z_row = f32(2.0) + vz0_t * T_GRID - f32(0.5) * f32(G) * (T_GRID * T_GRID)
    mlz = ((z_row < f32(3.0)) & (z_row > f32(0.0))).astype(f32)

    msk = (np.arange(NT) <= tof).astype(f32)       # tril row tof
    inv_am = f32(f32(1.0) / am)

    xin = np.zeros((P, XCOLS), f32)
    xin[:, _C_CXY:_C_CXY + 40] = cells[:, 0][None, :]
    xin[:, _C_CXY + 40:_C_CXY + 80] = cells[:, 1][None, :]
    xin[:, _C_RLXY:_C_RLXY + 40] = rlx[:, None]
    xin[:, _C_RLXY + 40:_C_RLXY + 80] = rly[:, None]
    xin[:, _C_RVXY:_C_RVXY + 40] = v_x_r[:, None]
    xin[:, _C_RVXY + 40:_C_RVXY + 80] = v_y_r[:, None]
    tgr = (T_GRID[TT_IDX] - reax).astype(f32)
    xin[:, _C_TG:_C_TG + 40] = np.where(mlz > 0, tgr, f32(-1e30))[None, :]
    xin[:, _C_MA:_C_MA + 40] = (msk * mlz)[None, :]   # col 0: shift==1 lane
    xin[:, _C_ONES:_C_ONES + P] = 1.0
    sc = _C_SC
    xin[:, sc + 0], xin[:, sc + 1] = sm, -sm
    xin[:, sc + 2], xin[:, sc + 3] = inv_am, -inv_am
    xin[:, sc + 4], xin[:, sc + 5] = f32(2.0) * am, -(f32(1.0) / sm)
    xin[:, sc + 6] = np.sqrt(f32(1.0) / (f32(2.0) * am * sm))
    xin[:, sc + 7] = sm * sm
    xin[:, sc + 8] = -f32(f32(3.14) / (f32(1.732) * sig))
    xin[:, sc + 9] = lo * teams + ld * (f32(1.0) - teams)
    return xin


def kernel(frame, s_max, a_max, tti_sigma, tti_lambda_off, tti_lambda_def):
    from concourse import bass_utils

    frame = np.asarray(frame, dtype=f32)
    B = frame.shape[0]
    nc = _get_nc()
    out = np.zeros((B, P), f32)
    for b in range(B):
        xin = _host_prep(frame[b:b + 1], s_max, a_max, tti_sigma,
                         tti_lambda_off, tti_lambda_def)
        in_maps = [{"xin": xin} for _ in range(N_CORES)]
        res = bass_utils.run_bass_kernel_spmd(nc, in_maps,
                                              core_ids=list(range(N_CORES)))
        out[b] = res.results[0]["res"][:, 0]
    return out
